# revision 16
# baseline (speedup 1.0000x reference)
"""MHA forward (B=4, N=1024, D=768, H=12, hd=64) on 8 TRN2 NeuronCores.

Sharding: tensor-parallel over heads x batch. Core c handles batch b=c//2 and
6 heads (first or second half by c%2). Each core computes its partial output
projection partial.T = w_proj[:, cols] @ ctx.T in DRAM; host sums the two
partials per batch and adds the bias.

On-core pipeline (all layouts chosen so no on-device transposes are needed):
  qT/kT  [head_dim, tok] = w{q,k}T.T @ xT          (per head, M=64)
  v      [tok, hd*6]     = xT.T @ wvT              (row-major, + ones col)
  m[q]   = max over first 128 keys of q.k (gpsimd cross-partition reduce)
  sT'    [key, q] = [kT; -1].T @ [qT; m]  (K=65 contraction folds -m[q] in)
  P.T    = exp(8*sT')  (ACT, scale=8 free affine)
  ctx.T  [hd+1, q] += [v | 1].T @ P.T    (row 64 accumulates l = sum_k P)
  ctx    normalized by 1/l (DVE reciprocal + gpsimd partition_broadcast)
  out.T  [768, q] += wpT.T @ ctx.T
Matmul operands are bitcast to float32r (1 cycle/row vs 4 for plain fp32).
"""

import numpy as np

import concourse.bass as bass
import concourse.bass_isa as bass_isa
import concourse.bacc as bacc
import concourse.mybir as mybir
from concourse.bass_utils import run_bass_kernel_spmd
from concourse.tile import TileContext

F32 = mybir.dt.float32
F32R = mybir.dt.float32r
U32 = mybir.dt.uint32
AF = mybir.ActivationFunctionType

B, N, D, H, HD = 4, 1024, 768, 12, 64
HPC = 6          # heads per core
NC = 8           # cores
SCALE = 8.0      # sqrt(HD); reference MULTIPLIES by it


def r32(ap):
    return ap.bitcast(F32R)


def build_nc():
    nc = bacc.Bacc()
    xT = nc.declare_dram_parameter("xT", [128, (D // 128) * N], F32R, isOutput=False)
    wqT = nc.declare_dram_parameter("wqT", [HPC, 128, (D // 128) * HD], F32R, isOutput=False)
    wkT = nc.declare_dram_parameter("wkT", [HPC, 128, (D // 128) * HD], F32R, isOutput=False)
    wvT = nc.declare_dram_parameter("wvT", [128, (D // 128) * HPC * HD], F32R, isOutput=False)
    wpT = nc.declare_dram_parameter("wpT", [HD, HPC * D], F32R, isOutput=False)
    outT = nc.declare_dram_parameter("outT", [D, N], F32, isOutput=True)

    DC = D // 128          # 6 contraction chunks over model dim
    KC = N // 128          # 8 key-row chunks
    QH = N // 512          # 2 query halves

    with TileContext(nc) as tc:
        with (
            tc.tile_pool(name="consts", bufs=1) as cpool,
            tc.tile_pool(name="qk", bufs=1) as qkpool,
            tc.tile_pool(name="va", bufs=1) as vapool,
            tc.tile_pool(name="work", bufs=2) as wpool,
            tc.tile_pool(name="pe", bufs=3) as pepool,
            tc.tile_pool(name="outsb", bufs=3) as opool,
            tc.tile_pool(name="mm", bufs=2, space="PSUM") as mmpool,
            tc.tile_pool(name="sps", bufs=2, space="PSUM") as spool,
            tc.tile_pool(name="cps0", bufs=2, space="PSUM") as cpool0,
            tc.tile_pool(name="cps1", bufs=2, space="PSUM") as cpool1,
        ):
            # ---- load constants (one DMA each to minimize sem fan-in) -----
            xtall = cpool.tile([128, DC * N], F32R, tag="xtall")
            nc.sync.dma_start(xtall[:], xT[:])
            xt = [xtall[:, N * i : N * (i + 1)] for i in range(DC)]
            wvall = cpool.tile([128, DC * HPC * HD], F32R, tag="wvall")
            nc.sync.dma_start(wvall[:], wvT[:])
            wv_sb = [wvall[:, HPC * HD * i : HPC * HD * (i + 1)] for i in range(DC)]
            wpall = cpool.tile([HD, HPC * D], F32R, tag="wpall")
            nc.sync.dma_start(wpall[:], wpT[:])
            wp_sb = [wpall[:, D * j : D * (j + 1)] for j in range(HPC)]
            biasc = cpool.tile([128, 1], F32, tag="biasc")
            nc.gpsimd.memset(biasc[:], -20.0)

            # ---- phase 1: qT/kT per head ([65, N]; row 64 = aug) ----------
            qa, ka = [], []
            for j in range(HPC):
                wq_t = wpool.tile([128, DC * HD], F32R, tag="wq")
                wk_t = wpool.tile([128, DC * HD], F32R, tag="wk")
                nc.sync.dma_start(wq_t[:], wqT[j])
                nc.sync.dma_start(wk_t[:], wkT[j])
                ta = qkpool.tile([65, N], F32R, tag=f"qa{j}")
                tb = qkpool.tile([65, N], F32R, tag=f"ka{j}")
                nc.gpsimd.memset(tb[64:65, :].bitcast(U32), 0xBF800000)  # -1.0f
                for t in range(QH):
                    ts = slice(512 * t, 512 * (t + 1))
                    psq = mmpool.tile([64, 512], F32, tag="mm")
                    psk = mmpool.tile([64, 512], F32, tag="mm")
                    for i in range(DC):
                        cs = slice(HD * i, HD * (i + 1))
                        nc.tensor.matmul(
                            psq[:], r32(wq_t[:, cs]), r32(xt[i][:, ts]),
                            start=(i == 0), stop=(i == DC - 1),
                        )
                    for i in range(DC):
                        cs = slice(HD * i, HD * (i + 1))
                        nc.tensor.matmul(
                            psk[:], r32(wk_t[:, cs]), r32(xt[i][:, ts]),
                            start=(i == 0), stop=(i == DC - 1),
                        )
                    nc.vector.tensor_copy(ta[0:64, ts], psq[:])
                    nc.vector.tensor_copy(tb[0:64, ts], psk[:])
                qa.append(ta)
                ka.append(tb)

            # ---- phase 1b: v row-major + ones col ([128, 65*HPC] per kc) --
            va = []
            for kc in range(KC):
                t = vapool.tile([128, 65 * HPC], F32R, tag=f"va{kc}")
                g65 = t[:].rearrange("p (h c) -> p h c", c=65)
                nc.gpsimd.memset(g65[:, :, 64:65].bitcast(U32), 0x3F800000)  # 1.0f
                ps = mmpool.tile([128, HPC * HD], F32, tag="mm")
                ks = slice(128 * kc, 128 * (kc + 1))
                for i in range(DC):
                    nc.tensor.matmul(
                        ps[:], r32(xt[i][:, ks]), r32(wv_sb[i]),
                        start=(i == 0), stop=(i == DC - 1),
                    )
                nc.vector.tensor_copy(
                    g65[:, :, 0:64],
                    ps[:].rearrange("p (h c) -> p h c", c=HD),
                )
                va.append(t)

            # ---- phase 2: attention per head ------------------------------
            ctxs = []
            for j in range(HPC):
                # subsample max over keys 0:128 -> qa row 64
                sub_sb = wpool.tile([128, N], F32, tag="ssub")
                for t in range(QH):
                    ts = slice(512 * t, 512 * (t + 1))
                    ps = mmpool.tile([128, 512], F32, tag="mm")
                    nc.tensor.matmul(
                        ps[:], r32(ka[j][0:64, 0:128]), r32(qa[j][0:64, ts]),
                        start=True, stop=True,
                    )
                    nc.vector.tensor_copy(sub_sb[:, ts], ps[:])
                par = wpool.tile([128, N], F32, tag="par")
                nc.gpsimd.partition_all_reduce(
                    par[:], sub_sb[:], 128, bass_isa.ReduceOp.max
                )
                nc.vector.tensor_copy(qa[j][64:65, :], par[64:65, :])

                c0 = cpool0.tile([65, 512], F32, tag="c0")
                c1 = cpool1.tile([65, 512], F32, tag="c1")
                cps = [c0, c1]
                for kc in range(KC):
                    ks = slice(128 * kc, 128 * (kc + 1))
                    pt = pepool.tile([128, N], F32R, tag="pe")
                    for t in range(QH):
                        ts = slice(512 * t, 512 * (t + 1))
                        ssp = spool.tile([128, 512], F32, tag="sps")
                        nc.tensor.matmul(
                            ssp[:], r32(ka[j][:, ks]), r32(qa[j][:, ts]),
                            start=True, stop=True,
                        )
                        # -20 bias: constant per-row shift (cancels in the
                        # normalization) that buys overflow headroom over the
                        # subsampled row max.
                        nc.scalar.activation(
                            pt[:, ts], ssp[:], AF.Exp, bias=biasc[:], scale=SCALE
                        )
                    for t in range(QH):
                        ts = slice(512 * t, 512 * (t + 1))
                        nc.tensor.matmul(
                            cps[t][:],
                            r32(va[kc][:, 65 * j : 65 * j + 65]),
                            r32(pt[:, ts]),
                            start=(kc == 0), stop=(kc == KC - 1),
                        )

                # normalize: ctx[0:64] * (1 / ctx[64])
                rec = wpool.tile([65, N], F32, tag="rec")
                rrec = wpool.tile([1, N], F32, tag="rrec")
                rbc = wpool.tile([64, N], F32, tag="rbc")
                ctx = qkpool.tile([64, N], F32R, tag=f"ctx{j}")
                for t in range(QH):
                    ts = slice(512 * t, 512 * (t + 1))
                    nc.vector.reciprocal(rec[64:65, ts], cps[t][64:65, :])
                # DMA shifts the 1/l row from partition 64 to partition 0
                nc.sync.dma_start(rrec[0:1, :], rec[64:65, :])
                nc.gpsimd.partition_broadcast(rbc[:], rrec[0:1, :])
                for t in range(QH):
                    ts = slice(512 * t, 512 * (t + 1))
                    nc.vector.tensor_mul(ctx[:, ts], cps[t][0:64, :], rbc[:, ts])
                ctxs.append(ctx)

            # ---- phase 3: output projection (partial, transposed) ---------
            for mt in range(DC):
                ms = slice(128 * mt, 128 * (mt + 1))
                for t in range(QH):
                    ts = slice(512 * t, 512 * (t + 1))
                    ps = mmpool.tile([128, 512], F32, tag="mm")
                    for j in range(HPC):
                        nc.tensor.matmul(
                            ps[:], r32(wp_sb[j][:, ms]), r32(ctxs[j][:, ts]),
                            start=(j == 0), stop=(j == HPC - 1),
                        )
                    osb = opool.tile([128, 512], F32, tag="osb")
                    nc.vector.tensor_copy(osb[:], ps[:])
                    nc.sync.dma_start(outT[ms, ts], osb[:])
    nc.finalize()
    return nc


_NC_CACHE = None


def _get_nc():
    global _NC_CACHE
    if _NC_CACHE is None:
        _NC_CACHE = build_nc()
    return _NC_CACHE


def make_in_maps(x, w_qkv, w_proj):
    x = np.asarray(x, dtype=np.float32)
    w_qkv = np.asarray(w_qkv, dtype=np.float32)
    in_maps = []
    for c in range(NC):
        b, hh = c // 2, c % 2
        h0 = HPC * hh
        def chunkT(a):
            # [D, m] -> [128, (D//128)*m]: d-chunk i lands at cols i*m:(i+1)*m
            m = a.shape[1]
            return np.ascontiguousarray(
                a.reshape(D // 128, 128, m).transpose(1, 0, 2).reshape(128, -1)
            )

        xTb = chunkT(x[b].T)                                     # [128, 6*N]
        wq = np.stack(
            [chunkT(w_qkv[HD * (h0 + j) : HD * (h0 + j + 1), :].T)
             for j in range(HPC)]
        )                                                        # [6, 128, 384]
        wk = np.stack(
            [chunkT(w_qkv[D + HD * (h0 + j) : D + HD * (h0 + j + 1), :].T)
             for j in range(HPC)]
        )
        wv = chunkT(w_qkv[2 * D + HD * h0 : 2 * D + HD * (h0 + HPC), :].T)
        wp = np.ascontiguousarray(
            np.stack(
                [w_proj[:, HD * (h0 + j) : HD * (h0 + j + 1)].T
                 for j in range(HPC)]
            ).transpose(1, 0, 2).reshape(HD, HPC * D)
        )                                                        # [64, 6*768]
        in_maps.append({"xT": xTb, "wqT": wq, "wkT": wk, "wvT": wv, "wpT": wp})
    return in_maps


def run(inputs, trace=False):
    nc = _get_nc()
    in_maps = make_in_maps(inputs["x"], inputs["w_qkv"], inputs["w_proj"])
    res = run_bass_kernel_spmd(nc, in_maps, list(range(NC)), trace=trace)
    b_proj = np.asarray(inputs["b_proj"], dtype=np.float32)
    out = np.empty((B, N, D), dtype=np.float32)
    for b in range(B):
        pT = res.results[2 * b]["outT"] + res.results[2 * b + 1]["outT"]
        out[b] = pT.T + b_proj[None, :]
    return out, res


def kernel(**inputs):
    return run(inputs)[0]
